# revision 102
# baseline (speedup 1.0000x reference)
import sys

sys.path.insert(0, "/opt/trn_rl_repo")
import numpy as np
import ml_dtypes

S, B, D, H = 1024, 4, 1024, 16
DH = D // H  # 64
HPC = 8  # heads per core
PG = HPC * DH  # 512 proj dims per core
KC = 9  # contraction chunks incl bias row (8 when biases all zero)
N_CORES = 8
ATT_SCALE = 1.0 / np.sqrt(DH)
NT = S // 128  # 8 t-chunks
NS = S // 512  # 2 s-tiles
VW = PG + HPC  # 520: vaug block width per t-chunk

_prog_cache = {}


def _build_program(kc=KC, dbg=False):
    import contextlib

    import concourse.tile as tile
    from concourse import bacc, mybir

    nc = bacc.Bacc(
        "TRN2",
        target_bir_lowering=False,
        debug=False,
        enable_asserts=False,
        num_devices=N_CORES,
    )
    f32 = mybir.dt.float32
    bf16 = mybir.dt.bfloat16
    f32r = mybir.dt.float32r
    EXP = mybir.ActivationFunctionType.Exp
    MUL = mybir.AluOpType.mult

    kpad = kc * 128
    xq = nc.dram_tensor("xq", (kpad, S), bf16, kind="ExternalInput").ap()
    xk = nc.dram_tensor("xk", (kpad, S), bf16, kind="ExternalInput").ap()
    xv = nc.dram_tensor("xv", (kpad, S), bf16, kind="ExternalInput").ap()
    # wq/wk pre-swizzled on host to SBUF layout: row pc*128+p holds partition
    # p's data for pc-block pc -> each pc piece is a plain [128, kc*128] copy.
    wq = nc.dram_tensor("wq", (4 * 128, kc * 128), bf16, kind="ExternalInput").ap()
    wk = nc.dram_tensor("wk", (4 * 128, kc * 128), bf16, kind="ExternalInput").ap()
    wv = nc.dram_tensor("wv", (kpad, PG), bf16, kind="ExternalInput").ap()
    wo = nc.dram_tensor("wo", (PG, D), bf16, kind="ExternalInput").ap()
    out = nc.dram_tensor("out", (S, D), bf16, kind="ExternalOutput").ap()
    if dbg:
        dbgt = {
            nm: nc.dram_tensor(nm, shp, bf16, kind="ExternalOutput").ap()
            for nm, shp in (
                ("dqp", (128, 4 * S)),
                ("dkp", (128, 4 * S)),
                ("dva", (128, NT * VW)),
                ("dcn", (128, 4 * S)),
            )
        }
        dca = {
            j: nc.dram_tensor(f"dca{j}", (128, S), mybir.dt.float32, kind="ExternalOutput").ap()
            for j in range(4)
        }
        drc = nc.dram_tensor("drc", (8, S), bf16, kind="ExternalOutput").ap()

    with tile.TileContext(nc) as tc:
        with contextlib.ExitStack() as ctx:
            persist = ctx.enter_context(tc.tile_pool(name="persist", bufs=1))
            xs = {
                n: persist.tile([128, kc * S], bf16, tag=f"x{n}", name=f"x_{n}")
                for n in "qkv"
            }
            ws = {
                n: persist.tile([128, kc * PG], bf16, tag=f"w{n}", name=f"w_{n}")
                for n in "qkv"
            }  # q/k are pc-major: pc block at pc*kc*128; v is kk-major
            qproj = persist.tile([128, 4 * S], bf16, tag="qproj")
            kproj = persist.tile([128, 4 * S], bf16, tag="kproj")
            vaug = persist.tile([128, NT * VW], bf16, tag="vaug")
            ctxn = persist.tile([128, 4 * S], bf16, tag="ctxn")
            onesb = persist.tile([128, 512], bf16, tag="onesb")
            wot = persist.tile([128, 4 * D], bf16, tag="wot")

            nc.vector.memset(onesb[:], 1.0)
            nc.vector.memset(vaug[:], 1.0)

            # DMA issue order = priority order (Tile scheduler honors it).
            # HWDGE has a ~625ns fixed cost per dma_start -> few, large
            # transfers; first-scores path (xq, wq-pc0, xk, wk-pc0) first.
            def load_x(nm, xap, a, b):
                nc.sync.dma_start(
                    xs[nm][:, a * S : b * S].rearrange("p (c n) -> p c n", c=b - a),
                    xap[a * 128 : b * 128, :].rearrange("(c p) n -> p c n", p=128),
                )

            def load_wpc(nm, wap, pc):
                nc.sync.dma_start(
                    ws[nm][:, pc * kc * 128 : (pc + 1) * kc * 128],
                    wap[pc * 128 : (pc + 1) * 128, :],
                )

            half = (kc + 1) // 2
            load_x("q", xq, 0, 2)
            load_wpc("q", wq, 0)
            load_x("q", xq, 2, half)
            load_x("q", xq, half, kc)
            load_wpc("q", wq, 1)
            load_x("k", xk, 0, half)
            load_wpc("k", wk, 0)
            load_x("k", xk, half, kc)
            load_wpc("k", wk, 1)
            load_wpc("q", wq, 2)
            load_wpc("q", wq, 3)
            load_wpc("k", wk, 2)
            load_wpc("k", wk, 3)
            load_x("v", xv, 0, kc)
            nc.sync.dma_start(
                ws["v"][:].rearrange("p (c n) -> p c n", c=kc),
                wv.rearrange("(c p) n -> p c n", p=128),
            )
            nc.sync.dma_start(
                wot[:].rearrange("p (c n) -> p c n", c=4),
                wo.rearrange("(c p) n -> p c n", p=128),
            )

            scp = ctx.enter_context(tc.tile_pool(name="scp", bufs=2, space="PSUM"))
            pvp = ctx.enter_context(tc.tile_pool(name="pvp", bufs=3, space="PSUM"))
            vnp = ctx.enter_context(tc.tile_pool(name="vnp", bufs=1, space="PSUM"))
            expp = ctx.enter_context(
                tc.tile_pool(name="expp", bufs=(16 if dbg else (28 if kc == 8 else 18)))
            )
            ctxp = ctx.enter_context(tc.tile_pool(name="ctxp", bufs=4 if dbg else 2))
            ptmpp = ctx.enter_context(tc.tile_pool(name="ptmp", bufs=4))
            outp = ctx.enter_context(tc.tile_pool(name="outp", bufs=3))
            recs = {
                (j, hh): persist.tile(
                    [128, S], bf16, tag=f"rcb{j}{hh}", name=f"rcb_{j}_{hh}"
                )
                for j in range(4)
                for hh in range(2)
            }

            # ---------- emission helpers (each yield = one PE matmul) ----------
            def gen_proj_qk(nm, dst, pc):
                accs = [
                    pvp.tile([128, 512], f32, tag="ps", name=f"acc_{nm}{pc}_{i}")
                    for i in range(2)
                ]
                wbase = pc * kc * 128
                for kk in range(kc):
                    for st in range(NS):
                        nc.tensor.matmul(
                            accs[st][:],
                            ws[nm][:, wbase + kk * 128 : wbase + (kk + 1) * 128],
                            xs[nm][:, kk * S + st * 512 : kk * S + st * 512 + 512],
                            start=(kk == 0),
                            stop=(kk == kc - 1),
                        )
                        yield
                for st in range(NS):
                    nc.vector.tensor_copy(
                        dst[:, pc * S + st * 512 : pc * S + st * 512 + 512],
                        accs[st][:],
                    )

            def gen_vblock(t, pool):
                acc = pool.tile([128, 512], f32, tag="ps", name=f"acc_v{t}")
                for kk in range(kc):
                    nc.tensor.matmul(
                        acc[:],
                        xs["v"][:, kk * S + t * 128 : kk * S + (t + 1) * 128],
                        ws["v"][:, kk * PG : (kk + 1) * PG],
                        start=(kk == 0),
                        stop=(kk == kc - 1),
                    )
                    yield
                vslice = vaug[:, t * VW : (t + 1) * VW]
                nc.vector.tensor_copy(
                    vslice.rearrange("p (h e) -> p h e", e=DH + 1)[:, :, 0:DH],
                    acc[:].rearrange("p (h e) -> p h e", h=HPC),
                )

            def gen_scores(j, exs, st_major=False):
                fo = j * S
                if st_major:
                    seq = [
                        (t, hh, st)
                        for st in range(NS)
                        for hh in range(2)
                        for t in range(NT)
                    ]
                else:
                    seq = [
                        (t, hh, st)
                        for t in range(NT)
                        for hh in range(2)
                        for st in range(NS)
                    ]
                scps = {}
                for t, hh, st in seq:
                    po = hh * 64
                    if t not in exs[hh]:
                        exs[hh][t] = expp.tile(
                            [128, S], bf16, tag="exp", name=f"ex{j}_{hh}_{t}"
                        )
                    et = exs[hh][t]
                    key = (t, hh, st) if st_major else (t, hh)
                    if key not in scps:
                        scps[key] = scp.tile(
                            [128, 1024], f32, tag="ps2", name=f"sc{j}_{t}_{hh}_{st}"
                        )
                    sc_ps = scps[key]
                    col = 0 if st_major else st * 512
                    nc.tensor.matmul(
                        sc_ps[:, col : col + 512],
                        kproj[po : po + 64, fo + t * 128 : fo + (t + 1) * 128],
                        qproj[po : po + 64, fo + st * 512 : fo + st * 512 + 512],
                        start=True,
                        stop=True,
                    )
                    if st_major:
                        # 512-wide activation (st halves produced far apart)
                        nc.scalar.activation(
                            et[:, st * 512 : st * 512 + 512], sc_ps[:, 0:512], EXP
                        )
                    elif st == NS - 1:
                        # one wide activation over both psum banks
                        nc.scalar.activation(et[:], sc_ps[:], EXP)
                    yield

            def pv_evict(j, hh, st, pv, ctxa):
                rcb = recs[(j, hh)]
                if hh == 0:
                    nc.vector.tensor_copy(
                        ctxa[0:64, st * 512 : st * 512 + 512], pv[0:64, :]
                    )
                else:
                    ptmp = ptmpp.tile(
                        [128, 512], f32, tag="ptmp", name=f"pt{j}_{hh}_{st}"
                    )
                    nc.vector.tensor_copy(ptmp[0:64, :], pv[0:64, :])
                    nc.sync.dma_start(
                        ctxa[64:128, st * 512 : st * 512 + 512],
                        ptmp[0:64, :],
                    )
                with nc.allow_low_precision("softmax denom reciprocal in bf16"):
                    nc.vector.reciprocal(
                        rcb[64:65, st * 512 : st * 512 + 512], pv[64:65, :]
                    )

            def pv_mm(j, hh, st, t, acc, exs):
                h = 2 * j + hh
                nc.tensor.matmul(
                    acc[0:65, :],
                    vaug[:, t * VW + h * (DH + 1) : t * VW + (h + 1) * (DH + 1)],
                    exs[hh][t][:, st * 512 : st * 512 + 512],
                    start=(t == 0),
                    stop=(t == NT - 1),
                )

            def gen_pv_st(j, exs, ctxa):
                # st-major: groups complete in order (st0,h0), (st0,h1), ...
                # with evictions folded in, so norm/outproj can chase per-st.
                for st in range(NS):
                    for hh in range(2):
                        acc = pvp.tile(
                            [128, 512], f32, tag="ps", name=f"pv{j}_{hh}_{st}"
                        )
                        for t in range(NT):
                            pv_mm(j, hh, st, t, acc, exs)
                            if t < NT - 1:
                                yield
                        pv_evict(j, hh, st, acc, ctxa)
                        yield

            def gen_norm(j, ctxa, sts=(0, 1)):
                fo = j * S
                for st in sts:
                    bc = vnp.tile([128, 512], f32, tag="ps", name=f"bc{j}_{st}")
                    nc.tensor.matmul(
                        bc[0:64, :],
                        onesb[64:65, 0:64],
                        recs[(j, 0)][64:65, st * 512 : st * 512 + 512],
                        start=True,
                        stop=True,
                        tile_position=(64, 0),
                    )
                    yield
                    nc.tensor.matmul(
                        bc[64:128, :],
                        onesb[64:65, 0:64],
                        recs[(j, 1)][64:65, st * 512 : st * 512 + 512],
                        start=True,
                        stop=True,
                        tile_position=(64, 64),
                    )
                    yield
                    nc.vector.tensor_tensor(
                        ctxn[:, fo + st * 512 : fo + st * 512 + 512],
                        ctxa[:, st * 512 : st * 512 + 512],
                        bc[:],
                        MUL,
                    )

            def gen_outproj():
                for sc in range(NT):
                    osb = outp.tile([128, D], bf16, tag="osb", name=f"osb_{sc}")
                    for nt in range(2):
                        acc = scp.tile(
                            [128, 1024], f32, tag="ps2", name=f"oacc_{sc}_{nt}"
                        )
                        for j in range(4):
                            nc.tensor.matmul(
                                acc[:, 0:512],
                                ctxn[:, j * S + sc * 128 : j * S + (sc + 1) * 128],
                                wot[:, j * D + nt * 512 : j * D + nt * 512 + 512],
                                start=(j == 0),
                                stop=(j == 3),
                            )
                            yield
                        nc.vector.tensor_copy(
                            osb[:, nt * 512 : nt * 512 + 512], acc[:, 0:512]
                        )
                        nc.sync.dma_start(
                            out[sc * 128 : (sc + 1) * 128, nt * 512 : nt * 512 + 512],
                            osb[:, nt * 512 : nt * 512 + 512],
                        )

            def drain(g):
                for _ in g:
                    pass

            def interleave(*streams):
                """streams: list of (gen, weight); emit weight items from each
                gen round-robin until all are exhausted."""
                streams = [[g, w] for g, w in streams]
                while streams:
                    done = []
                    for s in streams:
                        g, w = s
                        for _ in range(w):
                            try:
                                next(g)
                            except StopIteration:
                                done.append(s)
                                break
                    for s in done:
                        streams.remove(s)

            def chain(*gens):
                for g in gens:
                    yield from g

            # ---------- PE issue schedule ----------
            exs = [[None, None] for _ in range(4)]  # per pair: {hh: {t: tile}}
            for j in range(4):
                exs[j] = [{}, {}]
            ctxas = {}

            def get_ctxa(j):
                if j not in ctxas:
                    ctxas[j] = ctxp.tile([128, S], f32, tag="ctxa", name=f"ctxa_{j}")
                return ctxas[j]

            # PE p-state warmup: dummy matmuls while the first DMAs land
            # (results discarded; keeps pe_busy ramping to full clock)
            warm = vnp.tile([128, 512], f32, tag="ps", name="warm_ps")
            for _ in range(12):
                nc.tensor.matmul(
                    warm[0:64, :], onesb[:, 0:64], onesb[:], start=True, stop=True
                )

            # preamble: q pc0/pc1, k pc0 (matches DMA arrival order)
            drain(gen_proj_qk("q", qproj, 0))
            drain(gen_proj_qk("q", qproj, 1))
            drain(gen_proj_qk("k", kproj, 0))

            # S1: scores pair0 : remaining projections, 2:5
            interleave(
                (gen_scores(0, exs[0]), 2),
                (
                    chain(
                        gen_proj_qk("k", kproj, 1),
                        gen_proj_qk("q", qproj, 2),
                        gen_proj_qk("q", qproj, 3),
                        gen_proj_qk("k", kproj, 2),
                        gen_proj_qk("k", kproj, 3),
                    ),
                    5,
                ),
            )

            # S2: v-proj + pv0 : scores pair1  (3:1 -> sc1 spreads over the stream)
            def s2_stream():
                # all v-blocks emitted before any pv0 matmul: a pv0 matmul
                # emitted before its vaug block's producer would bind to the
                # stale memset contents (emission order defines dependencies)
                for t in range(NT):
                    yield from gen_vblock(t, vnp if t else pvp)
                yield from gen_pv_st(0, exs[0], get_ctxa(0))

            interleave((s2_stream(), 3), (gen_scores(1, exs[1]), 1))

            # S3: pv1 : scores pair2, 1:1; norm0 at stream end (no stall:
            # its inputs are long ready, and it only feeds the out-proj)
            interleave(
                (chain(gen_pv_st(1, exs[1], get_ctxa(1)), gen_norm(0, get_ctxa(0))), 1),
                (gen_scores(2, exs[2]), 1),
            )

            # S4: pv2 : scores pair3 (st-major), 1:1; norm1 at stream end
            interleave(
                (chain(gen_pv_st(2, exs[2], get_ctxa(2)), gen_norm(1, get_ctxa(1))), 1),
                (gen_scores(3, exs[3], st_major=True), 1),
            )

            # S5/S6 tail: pv3 st0 -> norm2+norm3 st0 -> outproj sc0-3
            # (interleaved with pv3 st1) -> norm3 st1 -> outproj sc4-7
            ctxa3 = get_ctxa(3)
            pv3 = gen_pv_st(3, exs[3], ctxa3)
            op = gen_outproj()
            for _ in range(16):  # pv3 (st0,h0)+(st0,h1) incl evictions
                next(pv3)
            drain(gen_norm(2, get_ctxa(2)))
            drain(gen_norm(3, ctxa3, sts=(0,)))

            def take(g, n):
                for _ in range(n):
                    try:
                        yield next(g)
                    except StopIteration:
                        return

            interleave((pv3, 1), (take(op, 32), 2))
            drain(gen_norm(3, ctxa3, sts=(1,)))
            drain(op)

            if dbg:
                for nm, t in (("dqp", qproj), ("dkp", kproj), ("dva", vaug), ("dcn", ctxn)):
                    nc.sync.dma_start(dbgt[nm][:, :], t[:])
                for jj in range(8):
                    nc.sync.dma_start(
                        drc[jj : jj + 1, :],
                        recs[(jj // 2, jj % 2)][64:65, :],
                    )
                for jj in range(4):
                    nc.sync.dma_start(dca[jj][:, :], ctxas[jj][:])

    nc.compile()
    return nc


def _get_program(kc=KC):
    if kc not in _prog_cache:
        _prog_cache[kc] = _build_program(kc)
    return _prog_cache[kc]


def _pad_k(a, kc=KC):
    """(1024, n) -> (kc*128, n); row 1024 = bias slot (set by caller) when kc=9."""
    if kc == 8:
        return np.ascontiguousarray(a, np.float32)
    p = np.zeros((kc * 128, a.shape[1]), np.float32)
    p[:D] = a
    return p


def _bf16(a):
    return np.ascontiguousarray(a.astype(ml_dtypes.bfloat16))


def _np_reference(q, k, v, attn_mask, Wq, bq, Wk, bk, Wv, bv, Wo, bo):
    def split_heads(x):
        return x.reshape(S, B, H, DH).transpose(2, 1, 0, 3)

    qh = split_heads(q @ Wq.T + bq)
    kh = split_heads(k @ Wk.T + bk)
    vh = split_heads(v @ Wv.T + bv)
    scores = np.einsum("hbsd,hbtd->hbst", qh, kh) * ATT_SCALE + attn_mask
    m = scores.max(-1, keepdims=True)
    e = np.exp(scores - m)
    probs = e / e.sum(-1, keepdims=True)
    ctx = np.einsum("hbst,hbtd->hbsd", probs, vh)
    ctx = ctx.transpose(2, 1, 0, 3).reshape(S, B, D)
    return (ctx @ Wo.T + bo).astype(np.float32)


def kernel(q, k, v, attn_mask, Wq, bq, Wk, bk, Wv, bv, Wo, bo, _want_results=False, _trace=False):
    q, k, v = (np.asarray(x, np.float32) for x in (q, k, v))
    attn_mask = np.asarray(attn_mask, np.float32)
    Wq, bq, Wk, bk, Wv, bv, Wo, bo = (
        np.asarray(x, np.float32) for x in (Wq, bq, Wk, bk, Wv, bv, Wo, bo)
    )
    if attn_mask.any():
        return _np_reference(q, k, v, attn_mask, Wq, bq, Wk, bk, Wv, bv, Wo, bo)

    from concourse import bass_utils

    zero_bias = not (bq.any() or bk.any() or bv.any())
    kc = 8 if zero_bias else 9
    nc = _get_program(kc)

    # permutation for wo rows: storage row (j, p) -> logical d = (2j + (p>=64))*64 + p%64
    p_idx = np.arange(128)
    perm = np.concatenate(
        [(2 * j + (p_idx >= 64)) * 64 + (p_idx % 64) for j in range(4)]
    )

    in_maps = []
    xT = {}
    for b in range(B):
        for nm, t in (("q", q), ("k", k), ("v", v)):
            a = _pad_k(np.ascontiguousarray(t[:, b, :].T), kc)
            if kc > 8:
                a[D] = 1.0  # bias row
            xT[(nm, b)] = _bf16(a)
    for c in range(N_CORES):
        b, g = c >> 1, c & 1
        cols = slice(g * PG, (g + 1) * PG)
        wqT = _pad_k(np.ascontiguousarray(Wq[cols].T) * ATT_SCALE, kc)
        wkT = _pad_k(np.ascontiguousarray(Wk[cols].T), kc)
        wvT = _pad_k(np.ascontiguousarray(Wv[cols].T), kc)
        if kc > 8:
            wqT[D] = bq[cols] * ATT_SCALE
            wkT[D] = bk[cols]
            wvT[D] = bv[cols]

        def pc_major(w):
            # (kpad, 512) -> (4*128, kc*128): row pc*128+p = partition p's
            # contiguous per-chunk data for pc-block pc (SBUF layout).
            kcc = w.shape[0] // 128
            return np.ascontiguousarray(
                w.reshape(kcc, 128, 4, 128).transpose(2, 1, 0, 3).reshape(4 * 128, kcc * 128)
            )

        wqT = pc_major(wqT)
        wkT = pc_major(wkT)
        woT = np.ascontiguousarray(Wo[:, cols].T)[perm]
        in_maps.append(
            {
                "xq": xT[("q", b)],
                "xk": xT[("k", b)],
                "xv": xT[("v", b)],
                "wq": _bf16(wqT),
                "wk": _bf16(wkT),
                "wv": _bf16(wvT),
                "wo": _bf16(woT),
            }
        )

    import tempfile

    kw = {}
    if _trace:
        kw = dict(trace=True, tmpdir=tempfile.mkdtemp(prefix="bass_trace_"))
    res = bass_utils.run_bass_kernel_spmd(nc, in_maps, core_ids=list(range(N_CORES)), **kw)
    out = np.empty((S, B, D), np.float32)
    for b in range(B):
        out[:, b, :] = (
            res.results[2 * b]["out"].astype(np.float32)
            + res.results[2 * b + 1]["out"].astype(np.float32)
            + bo
        )
    if _want_results:
        return out, res
    return out
